# revision 3
# baseline (speedup 1.0000x reference)
"""Trainium2 Bass kernel: 5x5 reflect-padded box-filter mean (LocalMean).

Full input:  image (32, 3, 512, 512) f32
Full output: same shape; out[r,c] = mean of the 5x5 window of the
reflect-padded image.

Strategy (pure data parallel over 8 NeuronCores, 4 images per core):
- Host pre-pads H and W by 2 with reflect -> (4, 3, 516, 516) per core.
- On-chip the filter is separable:
  * vertical 5-tap sum via TensorE banded matmuls (two constant weight
    tiles: D [128,128] main band, E [4,128] tail band; weights 1/25),
  * horizontal 5-tap sum via 3 DVE adds + 1 Pool add over shifted
    slices of the PSUM intermediate.
- DMA (HBM in 12.8MB + out 12.6MB per core) is the roofline bottleneck.
"""

import numpy as np

N_CORES = 8
B, C, H, W = 32, 3, 512, 512
PB = B // N_CORES          # images per core
PAD = 2
HP, WP = H + 2 * PAD, W + 2 * PAD   # 516

_CACHE = {}


def _band_weights():
    # D[k, m] = 1/25 for 0 <= k-m <= 4 (vertical 5-tap window starting at
    # output row m of the padded block); E covers the 4 tail rows that
    # spill into the next 128-row block.
    k = np.arange(128)[:, None]
    m = np.arange(128)[None, :]
    d = ((k - m >= 0) & (k - m <= 4)).astype(np.float32) / 25.0
    i = np.arange(4)[:, None]
    e = ((128 + i - m >= 0) & (128 + i - m <= 4)).astype(np.float32) / 25.0
    return d, e


def _build():
    import concourse.bacc as bacc
    import concourse.tile as tile
    from concourse import mybir

    f32 = mybir.dt.float32
    nc = bacc.Bacc("TRN2", target_bir_lowering=False, debug=False,
                   num_devices=N_CORES)
    x = nc.dram_tensor("x", [PB, C, HP, WP], f32, kind="ExternalInput").ap()
    wd = nc.dram_tensor("wd", [128, 128], f32, kind="ExternalInput").ap()
    we = nc.dram_tensor("we", [4, 128], f32, kind="ExternalInput").ap()
    y = nc.dram_tensor("y", [PB, C, H, W], f32, kind="ExternalOutput").ap()

    with tile.TileContext(nc) as tc:
        with (
            tc.tile_pool(name="wp", bufs=1) as wp,
            tc.tile_pool(name="xp", bufs=6) as xp,
            tc.tile_pool(name="xtp", bufs=2) as xtp,
            tc.tile_pool(name="vp", bufs=3, space="PSUM") as vp,
            tc.tile_pool(name="vsp", bufs=3) as vsp,
            tc.tile_pool(name="tp", bufs=3) as tp,
            tc.tile_pool(name="op", bufs=4) as op,
        ):
            d_t = wp.tile([128, 128], f32)
            nc.sync.dma_start(d_t[:], wd[:, :])
            e_t = wp.tile([4, 128], f32)
            nc.sync.dma_start(e_t[:], we[:, :])

            for n in range(PB):
                for c in range(C):
                    xts = []
                    for b in range(4):
                        t = xp.tile([128, WP], f32)
                        nc.sync.dma_start(t[:], x[n, c, 128 * b:128 * b + 128, :])
                        xts.append(t)
                    t4 = xtp.tile([4, WP], f32)
                    nc.sync.dma_start(t4[:], x[n, c, H:HP, :])
                    xts.append(t4)

                    for b in range(4):
                        v = vp.tile([128, WP], f32)
                        tail = xts[b + 1]
                        # V = D.T @ X_b + E.T @ X_tail, split at the PSUM
                        # bank boundary (fp32 matmul N <= 512).
                        nc.tensor.matmul(v[:, 0:512], d_t[:], xts[b][:, 0:512],
                                         start=True, stop=False)
                        nc.tensor.matmul(v[:, 512:516], d_t[:], xts[b][:, 512:516],
                                         start=True, stop=False)
                        nc.tensor.matmul(v[:, 0:512], e_t[:], tail[0:4, 0:512],
                                         start=False, stop=True)
                        nc.tensor.matmul(v[:, 512:516], e_t[:], tail[0:4, 512:516],
                                         start=False, stop=True)

                        # PSUM -> SBUF once (DVE/Pool ops may read at most
                        # one PSUM operand, Pool none), on the idle ScalarE.
                        vs = vsp.tile([128, WP], f32)
                        nc.scalar.copy(vs[:], v[:])
                        # Horizontal 5-tap: out = sum_{d=0..4} Vs[:, d:d+512]
                        t1 = tp.tile([128, W], f32)
                        nc.vector.tensor_add(t1[:], vs[:, 0:512], vs[:, 1:513])
                        t2 = tp.tile([128, W], f32)
                        nc.vector.tensor_add(t2[:], vs[:, 2:514], vs[:, 3:515])
                        t3 = tp.tile([128, W], f32)
                        nc.gpsimd.tensor_add(t3[:], t1[:], t2[:])
                        o = op.tile([128, W], f32)
                        nc.vector.tensor_add(o[:], t3[:], vs[:, 4:516])
                        nc.sync.dma_start(y[n, c, 128 * b:128 * b + 128, :], o[:])

    nc.compile()
    return nc


def _get_nc():
    if "nc" not in _CACHE:
        _CACHE["nc"] = _build()
    return _CACHE["nc"]


def _shard_inputs(image: np.ndarray):
    image = np.ascontiguousarray(np.asarray(image, dtype=np.float32))
    padded = np.pad(image, ((0, 0), (0, 0), (PAD, PAD), (PAD, PAD)),
                    mode="reflect")
    d, e = _band_weights()
    in_maps = []
    for i in range(N_CORES):
        in_maps.append({
            "x": np.ascontiguousarray(padded[i * PB:(i + 1) * PB]),
            "wd": d,
            "we": e,
        })
    return in_maps


def kernel(image: np.ndarray) -> np.ndarray:
    from concourse import bass_utils

    nc = _get_nc()
    in_maps = _shard_inputs(image)
    res = bass_utils.run_bass_kernel_spmd(nc, in_maps,
                                          core_ids=list(range(N_CORES)))
    return np.concatenate([res.results[i]["y"] for i in range(N_CORES)], axis=0)


# revision 6
# speedup vs baseline: 218.8971x; 218.8971x over previous
"""Trainium2 Bass kernel: 5x5 reflect-padded box-filter mean (LocalMean).

Full input:  image (32, 3, 512, 512) f32
Full output: same shape; out[r,c] = mean of the 5x5 window of the
reflect-padded image.

Strategy (pure data parallel over 8 NeuronCores, 4 images per core):
- Host pre-pads H and W by 2 with reflect -> (4, 3, 516, 516) per core.
- On-chip the filter is separable:
  * vertical 5-tap sum via TensorE banded matmuls (two constant weight
    tiles: D [128,128] main band, E [4,128] tail band; weights 1/25),
  * horizontal 5-tap sum via 3 DVE adds + 1 Pool add over shifted
    slices of the PSUM intermediate.
- DMA (HBM in 12.8MB + out 12.6MB per core) is the roofline bottleneck.
"""

import numpy as np

N_CORES = 8
B, C, H, W = 32, 3, 512, 512
PB = B // N_CORES          # images per core
PAD = 2
HP, WP = H + 2 * PAD, W + 2 * PAD   # 516

_CACHE = {}


def _band_weights():
    # D[k, m] = 1/25 for 0 <= k-m <= 4 (vertical 5-tap window starting at
    # output row m of the padded block); E covers the 4 tail rows that
    # spill into the next 128-row block.
    k = np.arange(128)[:, None]
    m = np.arange(128)[None, :]
    d = ((k - m >= 0) & (k - m <= 4)).astype(np.float32) / 25.0
    i = np.arange(4)[:, None]
    e = ((128 + i - m >= 0) & (128 + i - m <= 4)).astype(np.float32) / 25.0
    return d, e


def _build(reps=1):
    import concourse.bacc as bacc
    import concourse.tile as tile
    from concourse import mybir

    f32 = mybir.dt.float32
    nc = bacc.Bacc("TRN2", target_bir_lowering=False, debug=False,
                   num_devices=N_CORES)
    x = nc.dram_tensor("x", [PB, C, HP, WP], f32, kind="ExternalInput").ap()
    wd = nc.dram_tensor("wd", [128, 128], f32, kind="ExternalInput").ap()
    we = nc.dram_tensor("we", [4, 128], f32, kind="ExternalInput").ap()
    y = nc.dram_tensor("y", [PB, C, H, W], f32, kind="ExternalOutput").ap()

    with tile.TileContext(nc) as tc:
        with (
            tc.tile_pool(name="wp", bufs=1) as wp,
            tc.tile_pool(name="xp", bufs=6) as xp,
            tc.tile_pool(name="xtp", bufs=2) as xtp,
            tc.tile_pool(name="vp", bufs=3, space="PSUM") as vp,
            tc.tile_pool(name="vsp", bufs=3) as vsp,
            tc.tile_pool(name="tp", bufs=3) as tp,
            tc.tile_pool(name="op", bufs=4) as op,
        ):
            d_t = wp.tile([128, 128], f32)
            nc.sync.dma_start(d_t[:], wd[:, :])
            e_t = wp.tile([4, 128], f32)
            nc.sync.dma_start(e_t[:], we[:, :])

            for n, c in [(n, c) for _ in range(reps)
                         for n in range(PB) for c in range(C)]:
                if True:
                    xts = []
                    for b in range(4):
                        t = xp.tile([128, WP], f32)
                        nc.sync.dma_start(t[:], x[n, c, 128 * b:128 * b + 128, :])
                        xts.append(t)
                    t4 = xtp.tile([4, WP], f32)
                    nc.sync.dma_start(t4[:], x[n, c, H:HP, :])
                    xts.append(t4)

                    for b in range(4):
                        v = vp.tile([128, WP], f32)
                        tail = xts[b + 1]
                        # V = D.T @ X_b + E.T @ X_tail, split at the PSUM
                        # bank boundary (fp32 matmul N <= 512).
                        nc.tensor.matmul(v[:, 0:512], d_t[:], xts[b][:, 0:512],
                                         start=True, stop=False)
                        nc.tensor.matmul(v[:, 512:516], d_t[:], xts[b][:, 512:516],
                                         start=True, stop=False)
                        nc.tensor.matmul(v[:, 0:512], e_t[:], tail[0:4, 0:512],
                                         start=False, stop=True)
                        nc.tensor.matmul(v[:, 512:516], e_t[:], tail[0:4, 512:516],
                                         start=False, stop=True)

                        # PSUM -> SBUF once (DVE/Pool ops may read at most
                        # one PSUM operand, Pool none), on the idle ScalarE.
                        vs = vsp.tile([128, WP], f32)
                        nc.scalar.copy(vs[:], v[:])
                        # Horizontal 5-tap: out = sum_{d=0..4} Vs[:, d:d+512]
                        t1 = tp.tile([128, W], f32)
                        nc.vector.tensor_add(t1[:], vs[:, 0:512], vs[:, 1:513])
                        t2 = tp.tile([128, W], f32)
                        nc.vector.tensor_add(t2[:], vs[:, 2:514], vs[:, 3:515])
                        t3 = tp.tile([128, W], f32)
                        nc.gpsimd.tensor_add(t3[:], t1[:], t2[:])
                        o = op.tile([128, W], f32)
                        nc.vector.tensor_add(o[:], t3[:], vs[:, 4:516])
                        nc.sync.dma_start(y[n, c, 128 * b:128 * b + 128, :], o[:])

    nc.compile()
    return nc


def _get_nc(reps=1):
    key = ("nc", reps)
    if key not in _CACHE:
        _CACHE[key] = _build(reps)
    return _CACHE[key]


def _shard_inputs(image: np.ndarray):
    image = np.ascontiguousarray(np.asarray(image, dtype=np.float32))
    padded = np.pad(image, ((0, 0), (0, 0), (PAD, PAD), (PAD, PAD)),
                    mode="reflect")
    d, e = _band_weights()
    in_maps = []
    for i in range(N_CORES):
        in_maps.append({
            "x": np.ascontiguousarray(padded[i * PB:(i + 1) * PB]),
            "wd": d,
            "we": e,
        })
    return in_maps


def kernel(image: np.ndarray) -> np.ndarray:
    from concourse import bass_utils

    nc = _get_nc()
    in_maps = _shard_inputs(image)
    res = bass_utils.run_bass_kernel_spmd(nc, in_maps,
                                          core_ids=list(range(N_CORES)))
    return np.concatenate([res.results[i]["y"] for i in range(N_CORES)], axis=0)
